# revision 7
# baseline (speedup 1.0000x reference)
"""FConv2d via 9-tap matmul convolution on 8 TRN2 NeuronCores.

The reference computes ifft3(fft3(x) * fft3(W)) over a (128, 65, 65) grid,
crops, channel-subsamples by 4 and reshapes.  That is exactly:

  out[b, s*8+n, u, v] = sum_{dc<32, di<3, dj<3}
      W[n, dc, di, dj] * x_zp[b, (4s-dc) mod 128, u+1-di, v+1-dj]

(x_zp = x zero-padded by 1 spatially; the channel axis wraps circularly).
Per 3x3 tap this is a [256 x 128] channel-mixing matmul against a spatially
shifted view of x — a perfect TensorEngine job.  The tap matrices A are a
pure scatter of W (no arithmetic), built on host.

Sharding: data-parallel over batch B=8, one element per core.
"""

import numpy as np

import concourse.bass as bass
import concourse.tile as tile
from concourse import bacc, mybir
from concourse.bass_utils import run_bass_kernel_spmd

L = 64
CIN = 128
COUT = 256
NF = 8        # num filters
KS = 3        # kernel size
NTAP = KS * KS
B = 8
N_CORES = 8

# float32r: fp32 storage, 1 cycle/row matmul at N>=256 (vs 4 for float32).
# Walrus requires f32r matmul operands to be produced by a rounding op, so
# inputs are DMA'd as float32 and cast-copied into float32r tiles on DVE.
USE_F32R = True
MM_DT = mybir.dt.float32r if USE_F32R else mybir.dt.float32


def _build_A(W: np.ndarray) -> np.ndarray:
    """Scatter W [8, 32, 3, 3] into tap matrices A [128, 9*256] (f32).

    A[c, t*256 + co] = W[co%8, (4*(co//8) - c) % 128, 2-e, 2-f], t = e*3+f,
    zero where the channel offset is outside [0, 32).
    """
    A = np.zeros((CIN, NTAP, COUT), np.float32)
    c = np.arange(CIN)
    for co in range(COUT):
        s, n = co // NF, co % NF
        dc = (4 * s - c) % CIN
        mask = dc < 32
        for e in range(KS):
            for f in range(KS):
                A[mask, e * KS + f, co] = W[n, dc[mask], 2 - e, 2 - f]
    return np.ascontiguousarray(A.reshape(CIN, NTAP * COUT))


def _build_program():
    nc = bacc.Bacc("TRN2", target_bir_lowering=False, debug=False,
                   num_devices=N_CORES)
    x_ap = nc.dram_tensor("x", [CIN, L, L], mybir.dt.float32,
                          kind="ExternalInput").ap()
    a_ap = nc.dram_tensor("A", [CIN, NTAP * COUT], mybir.dt.float32,
                          kind="ExternalInput").ap()
    out_ap = nc.dram_tensor("out", [COUT, L, L], mybir.dt.float32,
                            kind="ExternalOutput").ap()

    with tile.TileContext(nc) as tc:
        with (
            tc.tile_pool(name="const", bufs=1) as const_pool,
            tc.tile_pool(name="psum", bufs=4, space="PSUM") as psum_pool,
            tc.tile_pool(name="outs", bufs=4) as out_pool,
        ):
            A_raw = const_pool.tile([CIN, NTAP * COUT], mybir.dt.float32)
            nc.sync.dma_start(A_raw[:], a_ap[:])
            A_sb = const_pool.tile([CIN, NTAP * COUT], MM_DT)
            nc.vector.tensor_copy(A_sb[:], A_raw[:])

            xs = const_pool.tile([CIN, L, L], mybir.dt.float32)
            nc.sync.dma_start(xs[:], x_ap[:])
            zrow = const_pool.tile([CIN, L + 2], mybir.dt.float32)
            nc.vector.memset(zrow[:], 0.0)

            # x zero-padded by 1 on each spatial side: [128, 66, 66]
            xp = const_pool.tile([CIN, L + 2, L + 2], MM_DT)
            nc.vector.tensor_copy(xp[:, 0, :], zrow[:])
            nc.vector.tensor_copy(xp[:, L + 1, :], zrow[:])
            nc.vector.tensor_copy(xp[:, :, 0], zrow[:])
            nc.vector.tensor_copy(xp[:, :, L + 1], zrow[:])
            nc.vector.tensor_copy(xp[:, 1:L + 1, 1:L + 1], xs[:])

            ROWS = 8                       # output rows per spatial chunk
            NQ = L // ROWS                 # 8 chunks
            for h in range(2):             # output-channel halves
                for q in range(NQ):
                    ps = psum_pool.tile([128, ROWS * L], mybir.dt.float32)
                    for t in range(NTAP):
                        e, f = t // KS, t % KS
                        lhsT = A_sb[:, t * COUT + h * 128:
                                    t * COUT + h * 128 + 128]
                        rhs = xp[:, ROWS * q + e:ROWS * q + e + ROWS,
                                 f:f + L]
                        nc.tensor.matmul(ps[:], lhsT, rhs,
                                         start=(t == 0), stop=(t == NTAP - 1))
                    o = out_pool.tile([128, ROWS * L], mybir.dt.float32)
                    nc.vector.tensor_copy(o[:], ps[:])
                    nc.sync.dma_start(
                        out_ap[h * 128:h * 128 + 128,
                               ROWS * q:ROWS * q + ROWS, :],
                        o[:].rearrange("p (a b) -> p a b", a=ROWS))
    nc.compile()
    return nc


_PROGRAM = None


def _get_program():
    global _PROGRAM
    if _PROGRAM is None:
        _PROGRAM = _build_program()
    return _PROGRAM


def kernel(x: np.ndarray, W: np.ndarray) -> np.ndarray:
    x = np.ascontiguousarray(np.asarray(x, dtype=np.float32))
    W = np.asarray(W, dtype=np.float32)
    A = _build_A(W)
    nc = _get_program()
    in_maps = [{"x": np.ascontiguousarray(x[b]), "A": A} for b in range(B)]
    res = run_bass_kernel_spmd(nc, in_maps, list(range(N_CORES)))
    return np.stack([res.results[i]["out"] for i in range(N_CORES)], axis=0)


# revision 8
# speedup vs baseline: 1.0963x; 1.0963x over previous
"""FConv2d via 9-tap matmul convolution on 8 TRN2 NeuronCores.

The reference computes ifft3(fft3(x) * fft3(W)) over a (128, 65, 65) grid,
crops, channel-subsamples by 4 and reshapes.  That is exactly:

  out[b, s*8+n, u, v] = sum_{dc<32, di<3, dj<3}
      W[n, dc, di, dj] * x_zp[b, (4s-dc) mod 128, u+1-di, v+1-dj]

(x_zp = x zero-padded by 1 spatially; the channel axis wraps circularly).
Per 3x3 tap this is a [256 x 128] channel-mixing matmul against a spatially
shifted view of x — a perfect TensorEngine job.  The tap matrices A are a
pure scatter of W (no arithmetic), built on host.

Sharding: data-parallel over batch B=8, one element per core.

Schedule notes:
- matmuls in float32r (fp32 storage, ~1 cyc/col at N=512 vs 4 for fp32);
  walrus requires f32r operands to come from a rounding producer, so inputs
  are DMA'd as f32 and cast-copied on DVE.
- x is DMA'd in 4 chunks on the sync HWDGE ring while A halves ride the
  scalar HWDGE ring (two physical rings -> parallel HBM streams); casts
  chase chunk completions so the PE starts ~10us earlier than a monolithic
  load would allow.
"""

import numpy as np

import concourse.bass as bass
import concourse.tile as tile
from concourse import bacc, mybir
from concourse.bass_utils import run_bass_kernel_spmd

L = 64
CIN = 128
COUT = 256
NF = 8        # num filters
KS = 3        # kernel size
NTAP = KS * KS
B = 8
N_CORES = 8

USE_F32R = True
MM_DT = mybir.dt.float32r if USE_F32R else mybir.dt.float32

NXCHUNK = 4                 # x DMA chunks (rows per chunk = L / NXCHUNK)
XROWS = L // NXCHUNK
HALF = NTAP * 128           # A columns per output-channel half


def _build_A(W: np.ndarray) -> np.ndarray:
    """Scatter W [8, 32, 3, 3] into tap matrices A [128, 2*9*128] (f32).

    Layout: A[c, h*1152 + t*128 + m] = W[n, (4s-c)%128, 2-e, 2-f] where
    co = h*128 + m, s = co//8, n = co%8, t = e*3+f; zero outside the
    32-channel window.  Each output half h is contiguous so it can be
    DMA'd (and cast) independently.
    """
    A = np.zeros((CIN, 2, NTAP, 128), np.float32)
    c = np.arange(CIN)
    for co in range(COUT):
        s, n = co // NF, co % NF
        h, m = co // 128, co % 128
        dc = (4 * s - c) % CIN
        mask = dc < 32
        for e in range(KS):
            for f in range(KS):
                A[mask, h, e * KS + f, m] = W[n, dc[mask], 2 - e, 2 - f]
    return np.ascontiguousarray(A.reshape(CIN, 2 * HALF))


def _build_program():
    nc = bacc.Bacc("TRN2", target_bir_lowering=False, debug=False,
                   num_devices=N_CORES)
    x_ap = nc.dram_tensor("x", [CIN, L, L], mybir.dt.float32,
                          kind="ExternalInput").ap()
    a_ap = nc.dram_tensor("A", [CIN, 2 * HALF], mybir.dt.float32,
                          kind="ExternalInput").ap()
    out_ap = nc.dram_tensor("out", [COUT, L, L], mybir.dt.float32,
                            kind="ExternalOutput").ap()

    with tile.TileContext(nc) as tc:
        with (
            tc.tile_pool(name="const", bufs=1) as const_pool,
            tc.tile_pool(name="psum", bufs=4, space="PSUM") as psum_pool,
            tc.tile_pool(name="outs", bufs=4) as out_pool,
        ):
            # --- input staging -------------------------------------------
            # x chunks on the sync HWDGE ring; A halves on the scalar ring.
            xs = const_pool.tile([CIN, L, L], mybir.dt.float32)
            for k in range(NXCHUNK):
                nc.sync.dma_start(xs[:, XROWS * k:XROWS * (k + 1), :],
                                  x_ap[:, XROWS * k:XROWS * (k + 1), :])

            A_raw = const_pool.tile([CIN, 2 * HALF], mybir.dt.float32)
            A_sb = const_pool.tile([CIN, 2 * HALF], MM_DT)
            for h in range(2):
                nc.scalar.dma_start(A_raw[:, h * HALF:(h + 1) * HALF],
                                    a_ap[:, h * HALF:(h + 1) * HALF])
                nc.vector.tensor_copy(A_sb[:, h * HALF:(h + 1) * HALF],
                                      A_raw[:, h * HALF:(h + 1) * HALF])

            # x zero-padded by 1 on each spatial side, rounded to f32r.
            zrow = const_pool.tile([CIN, L + 2], mybir.dt.float32)
            nc.vector.memset(zrow[:], 0.0)
            xp = const_pool.tile([CIN, L + 2, L + 2], MM_DT)
            nc.vector.tensor_copy(xp[:, 0, :], zrow[:])
            nc.vector.tensor_copy(xp[:, L + 1, :], zrow[:])
            nc.vector.tensor_copy(xp[:, :, 0], zrow[:])
            nc.vector.tensor_copy(xp[:, :, L + 1], zrow[:])
            for k in range(NXCHUNK):
                nc.vector.tensor_copy(
                    xp[:, 1 + XROWS * k:1 + XROWS * (k + 1), 1:L + 1],
                    xs[:, XROWS * k:XROWS * (k + 1), :])

            # --- 9-tap matmul conv ---------------------------------------
            ROWS = 8                       # output rows per spatial chunk
            NQ = L // ROWS
            for h in range(2):             # output-channel halves
                for q in range(NQ):
                    ps = psum_pool.tile([128, ROWS * L], mybir.dt.float32)
                    for t in range(NTAP):
                        e, f = t // KS, t % KS
                        lhsT = A_sb[:, h * HALF + t * 128:
                                    h * HALF + t * 128 + 128]
                        rhs = xp[:, ROWS * q + e:ROWS * q + e + ROWS,
                                 f:f + L]
                        nc.tensor.matmul(ps[:], lhsT, rhs,
                                         start=(t == 0), stop=(t == NTAP - 1))
                    o = out_pool.tile([128, ROWS * L], mybir.dt.float32)
                    nc.vector.tensor_copy(o[:], ps[:])
                    nc.sync.dma_start(
                        out_ap[h * 128:h * 128 + 128,
                               ROWS * q:ROWS * q + ROWS, :],
                        o[:].rearrange("p (a b) -> p a b", a=ROWS))
    nc.compile()
    return nc


_PROGRAM = None


def _get_program():
    global _PROGRAM
    if _PROGRAM is None:
        _PROGRAM = _build_program()
    return _PROGRAM


def kernel(x: np.ndarray, W: np.ndarray) -> np.ndarray:
    x = np.ascontiguousarray(np.asarray(x, dtype=np.float32))
    W = np.asarray(W, dtype=np.float32)
    A = _build_A(W)
    nc = _get_program()
    in_maps = [{"x": np.ascontiguousarray(x[b]), "A": A} for b in range(B)]
    res = run_bass_kernel_spmd(nc, in_maps, list(range(N_CORES)))
    return np.stack([res.results[i]["out"] for i in range(N_CORES)], axis=0)
